# revision 45
# baseline (speedup 1.0000x reference)
"""NonLocalBlock (embedded-gaussian attention) TRN2 kernel, v2.

Shapes (hardcoded): x [8, 256, 64, 64] fp32, one batch element per core.
Per core:
  theta/phi/g = 1x1 conv projections of x_b [256, 4096] -> [128, 4096]
  f^T[j, i] = sum_c phi[c, j] theta[c, i]        (4096 x 4096 logits)
  ef = exp(f - 40) in bf16 (shift cancels in y/Z; avoids bf16 overflow)
  y[ci, i] = sum_j ef[j, i] gT[j, ci]            (bf16 matmuls, fp32 PSUM)
  Z[i] = sum_j ef[j, i]  via bf16 adds on DVE (binary-counter tree for
         j<24, then in-place running adds so only one DVE op trails the
         last exp), closed by a ones-matmul partition reduce on PE
  out = x + W_w @ (y / Z) + (W_w @ g_b + W_b)    (g bias folded, sum soft=1)

Steady state is ScalarE-bound (exp of 16.7M elems at ~1117 ns per 1024-col
tile; measured main loop = 128 x 1117 ns, Scalar ~100% busy).  Everything
else is kept under that cadence:
  - PE per t: f 2x512-col f32r + y 2x512-col bf16 ~ 930 ns; Z is NOT on PE
    (except 2 closing 512-col ones-MMs per quarter).
  - DVE per t: Z adds (bf16 2x mode, ~690 ns each) + piece evictions.
  - GpSimd only does the 1/Z partition broadcasts (its slow tensor ops
    share the DVE SBUF port and starve 2-port DVE instructions).
  - x is DMAed in column chunks on both hw queues (the rings ramp slowly:
    ~60 GB/s cold, so the first 512-col chunk is split out to land early);
    projections run as 512-col pieces: chunks 0/1 pre-loop, the rest
    injected into main-loop PE slack with a deadline-based schedule
    (in-loop PSUM pieces share the pw tag with W-proj/Z-close tiles).
  - Warmup matmuls (plus a few interleaved between pre-loop pieces) and a
    dummy activation bridge the DMA wait so the HAM clock gate never drops
    to 1.2 GHz and the exp table load is off the critical path.
  - Per-quarter deferred ops in the next quarter: Z-close j=3,4; recip
    (fast, ~18 bits) j=5; broadcast j=6,7; normalize j=11 (= YLAG, first
    slot after the y accumulation group closes); W-projection j=12..15.
  - Tail: dummy matmuls keep the PE clock warm through the Z-finalize
    chain; final out-DMAs alternate both queues.
"""

import numpy as np

import concourse.bacc as bacc
import concourse.mybir as mybir
from concourse import tile
from concourse.bass_utils import run_bass_kernel_spmd

F32 = mybir.dt.float32
F32R = mybir.dt.float32r
BF16 = mybir.dt.bfloat16
AF = mybir.ActivationFunctionType

B, C, CI = 8, 256, 128
H, Wd = 64, 64
N = H * Wd              # 4096
NQ = 4                  # i-quarters
QW = N // NQ            # 1024
JB = N // 128           # 32 j-blocks (= ts per quarter)
T = NQ * JB             # 128
NCH = 8                 # 512-col x pieces for projections
PW = N // NCH           # 512

YLAG = 11               # y-matmul lag behind f/exp
ZCLOSE_J = (3, 4)       # prev quarter's Z partition-reduce MMs
ZINV_J = 5              # prev quarter's reciprocals
BCAST_J = (6, 7)        # prev quarter's Z broadcast (gpsimd)
MULT_J = 11             # prev quarter's normalize (= YLAG: y-accum group
                        # closes at j=YLAG-1, so this is the first safe slot)
WPROJ_J0 = 12           # prev quarter's W-projection chunks (4)
EXP_BIAS = -40.0
NWARM = 11              # warmup matmuls: bridge PE activity from the start
                        # barrier (~7.4us) to x-chunk-A1 landing (~11.3us) so
                        # the HAM clock gate never drops to 1.2 GHz
ZRUN_J = 24             # switch Z accumulation from tree to running adds here


def _build_sched():
    # t -> list of ops. pieces are 512-col units c=0..7 of x columns.
    # deadlines: ph piece c before f uses j-block 4c (t=4c); th piece c
    # before f of quarter c//2 (t=32*(c//2)); g piece c before y uses
    # block 4c (t=YLAG+4c).
    sched = {}

    def add(t, op):
        sched.setdefault(t, []).append(op)

    add(1, ("ph", 1))
    for c in range(2, 8):
        add(4 * c - 3, ("ph", c))
    for c, t in ((2, 24), (3, 26), (4, 48), (5, 52), (6, 80), (7, 84)):
        add(t, ("th", c))
    gsched = {0: 2, 1: 6, 2: 10, 3: 14, 4: 18, 5: 22, 6: 30, 7: 33}
    for c, t in gsched.items():
        add(t, ("ga", c))
        add(t + 1, ("gb", c))
    # bias add for chunk c after last of {th_c, ph_c, gb_c}
    for c, t in ((0, 4), (1, 8), (2, 25), (3, 27), (4, 49), (5, 53),
                 (6, 81), (7, 85)):
        add(t, ("bias", c))
    return sched


def build():
    nc = bacc.Bacc("TRN2", target_bir_lowering=False, debug=False, num_devices=8)

    x_d = nc.dram_tensor("x", [C, N], F32R, kind="ExternalInput")
    thw_d = nc.dram_tensor("thw_t", [C, CI], F32R, kind="ExternalInput")  # theta_w.T
    phw_d = nc.dram_tensor("phw_t", [C, CI], F32R, kind="ExternalInput")  # phi_w.T
    gw_d = nc.dram_tensor("gw_t", [C, CI], F32R, kind="ExternalInput")    # g_w.T
    ww_d = nc.dram_tensor("ww_t", [CI, C], F32R, kind="ExternalInput")    # W_w.T
    # aux cols: 0=theta_b, 1=phi_b, 2=wb_eff[:128], 3=wb_eff[128:], 4=ones,
    # 5=exp bias (-40)
    aux_d = nc.dram_tensor("aux", [128, 6], F32, kind="ExternalInput")
    out_d = nc.dram_tensor("out", [C, N], F32, kind="ExternalOutput")

    sched = _build_sched()

    with tile.TileContext(nc) as tc:
        with (
            tc.tile_pool(name="const", bufs=1) as cpool,
            tc.tile_pool(name="big", bufs=1) as bigpool,
            tc.tile_pool(name="ef", bufs=13) as efpool,
            tc.tile_pool(name="ztree", bufs=2) as ztpool,
            tc.tile_pool(name="zpool", bufs=2) as zpool,
            tc.tile_pool(name="ypool", bufs=2) as ypool,
            tc.tile_pool(name="opool", bufs=6) as opool,
            tc.tile_pool(name="pf", bufs=2, space="PSUM") as pf,
            tc.tile_pool(name="py", bufs=1, space="PSUM") as py,
            tc.tile_pool(name="pw", bufs=2, space="PSUM") as pw,
        ):
            # ---------------- warmup + DMA issue ----------------
            warm = cpool.tile([128, 512], BF16, tag="warm")
            warm2 = cpool.tile([128, 1], F32, tag="warm2")
            nc.gpsimd.memset(warm[:], 0.0)

            aux = cpool.tile([128, 6], F32, tag="aux")
            thw = cpool.tile([128, 2 * CI], F32R, tag="thw")
            phw = cpool.tile([128, 2 * CI], F32R, tag="phw")
            gw = cpool.tile([128, 2 * CI], F32R, tag="gw")
            ww = cpool.tile([CI, C], F32R, tag="ww")
            x0 = bigpool.tile([128, N], F32R, tag="x0")
            x1 = bigpool.tile([128, N], F32R, tag="x1")
            xs = (x0, x1)

            # x chunk A first on both queues (the DMA rings are slow to ramp;
            # chunk A gates the first projections), then weights.  The dummy
            # activation pulls the ~2.7us exp table load off the critical
            # path, but only after the two most urgent scalar-queue issues.
            # first x chunk split in two so the 0:512 half (which gates the
            # first projection pieces) lands ~4us earlier on the cold rings
            nc.sync.dma_start(x0[:, 0:512], x_d[0:128, 0:512])
            nc.scalar.dma_start(x1[:, 0:512], x_d[128:256, 0:512])
            nc.sync.dma_start(thw[:, 0:CI], thw_d[0:128, :])
            nc.scalar.dma_start(thw[:, CI:2 * CI], thw_d[128:256, :])
            nc.scalar.activation(warm2[:], warm[:, 0:1], AF.Identity)
            nc.sync.dma_start(aux[:], aux_d[:])
            nc.sync.dma_start(phw[:, 0:CI], phw_d[0:128, :])
            nc.scalar.dma_start(phw[:, CI:2 * CI], phw_d[128:256, :])
            nc.sync.dma_start(gw[:, 0:CI], gw_d[0:128, :])
            nc.scalar.dma_start(gw[:, CI:2 * CI], gw_d[128:256, :])
            nc.sync.dma_start(x0[:, 512:1024], x_d[0:128, 512:1024])
            nc.scalar.dma_start(x1[:, 512:1024], x_d[128:256, 512:1024])

            for _ in range(NWARM):
                pwt = pw.tile([128, 512], F32, tag="pw", name="warm_mm")
                nc.tensor.matmul(pwt[:], warm[:, 0:128], warm[:],
                                 start=True, stop=True)

            thb, phb = aux[:, 0:1], aux[:, 1:2]
            wbe = (aux[:, 2:3], aux[:, 3:4])
            ones_bf = cpool.tile([128, 1], BF16, tag="ones_bf")
            nc.vector.tensor_copy(ones_bf[:], aux[:, 4:5])

            th_sb = bigpool.tile([128, N], F32R, tag="th")
            ph_sb = bigpool.tile([128, N], F32R, tag="ph")
            gT_sb = bigpool.tile([128, N], BF16, tag="gT")

            # ---------------- projection piece emitters ----------------
            def proj_piece(kind, c, ev="v", pool_tag="pw"):
                # kind in ("th", "ph"): [ci, 512] piece of theta/phi.
                # ev: "v" DVE eviction (in-loop: ScalarE is the exp
                # bottleneck), "s" ScalarE (pre-loop parallelism).
                # pool_tag "pf" pre-loop avoids pw-rotation serialization.
                wt, bias_t, dst = ((thw, thb, th_sb) if kind == "th"
                                   else (phw, phb, ph_sb))
                lo = c * PW
                pool = pf if pool_tag == "pf" else pw
                pp = pool.tile([128, 512], F32, tag=pool_tag,
                               name=f"{kind}p_{c}")
                for k in range(2):
                    nc.tensor.matmul(
                        pp[:], wt[:, k * CI:(k + 1) * CI],
                        xs[k][:, lo:lo + 512],
                        start=(k == 0), stop=(k == 1),
                    )
                if ev == "s":
                    nc.scalar.activation(dst[:, lo:lo + 512], pp[:],
                                         AF.Identity, bias=bias_t)
                else:
                    nc.vector.tensor_scalar_add(dst[:, lo:lo + 512], pp[:],
                                                bias_t)

            gtiles = {}

            def g_piece(c, half):
                # gT blocks 4c+2*half, 4c+2*half+1 into shared [128,512] tile
                if half == 0:
                    gtiles[c] = pw.tile([128, 512], F32, tag="pw",
                                        name=f"gp_{c}")
                pg = gtiles[c]
                for b in (4 * c + 2 * half, 4 * c + 2 * half + 1):
                    col = (b - 4 * c) * 128
                    for k in range(2):
                        nc.tensor.matmul(
                            pg[:, col:col + 128],
                            xs[k][:, b * 128:(b + 1) * 128],
                            gw[:, k * CI:(k + 1) * CI],
                            start=(k == 0), stop=(k == 1),
                        )
                if half == 1:
                    lo = c * PW
                    nc.vector.tensor_copy(gT_sb[:, lo:lo + 512], pg[:])
                    del gtiles[c]

            def bias_chunk(c):
                # DVE, not GpSimd: GpSimd shares its SBUF port with the DVE
                # and its slow tensor ops starve 2-port DVE instructions.
                lo = c * PW
                for k in range(2):
                    nc.vector.tensor_scalar_add(
                        xs[k][:, lo:lo + 512], xs[k][:, lo:lo + 512], wbe[k])

            def emit_sched_op(op):
                kind = op[0]
                if kind in ("th", "ph"):
                    proj_piece(kind, op[1])
                elif kind == "ga":
                    g_piece(op[1], 0)
                elif kind == "gb":
                    g_piece(op[1], 1)
                elif kind == "bias":
                    bias_chunk(op[1])

            # preloop: th chunks 0,1 + ph chunk 0.  A1-gated pieces first
            # (th0, ph0), A2-gated th1 last; evictions split scalar/DVE so
            # they pipeline.  Warmup matmuls interleave into the dead PE
            # windows (waiting for x-A2 / evictions) so the HAM activity
            # monitor never sees a ~3.4us idle window and re-throttles.
            def pre_warm(n):
                for _ in range(n):
                    pwt = pw.tile([128, 512], F32, tag="pw", name="warm_mm")
                    nc.tensor.matmul(pwt[:], warm[:, 0:128], warm[:],
                                     start=True, stop=True)

            proj_piece("th", 0, ev="s")
            proj_piece("ph", 0, ev="v")
            # pre-emit f(0)'s s=0 half: its data (th chunk 0 + ph block 0)
            # is ready ~1.5us before the A2-gated th1 matmuls ahead of it in
            # the Tensor FIFO; the loop below emits only the s=1 half.
            pf0 = pf.tile([128, QW], F32, tag="pf", name="pf_0")
            nc.tensor.matmul(pf0[:, 0:512], ph_sb[:, 0:128], th_sb[:, 0:512],
                             start=True, stop=True)
            pre_warm(7)
            proj_piece("th", 1, ev="s")
            pre_warm(3)

            # deferred x DMA chunks
            nc.sync.dma_start(x0[:, 1024:2048], x_d[0:128, 1024:2048])
            nc.scalar.dma_start(x1[:, 1024:2048], x_d[128:256, 1024:2048])
            nc.sync.dma_start(x0[:, 2048:4096], x_d[0:128, 2048:4096])
            nc.scalar.dma_start(x1[:, 2048:4096], x_d[128:256, 2048:4096])
            nc.sync.dma_start(ww[:], ww_d[:])

            # ---------------- per-quarter deferred ops ----------------
            state = {}   # per-quarter: zq, pzt[2], zi, zb, pyt, ynt
            efs = {}

            def zclose(q, s):
                st = state[q]
                pzt = pw.tile([1, 512], F32, tag="pw", name=f"pz_{q}_{s}")
                st["pzt"][s] = pzt
                nc.tensor.matmul(pzt[:], ones_bf[:],
                                 st["zq"][:, s * 512:(s + 1) * 512],
                                 start=True, stop=True)

            def zinv(q, s):
                # fast variant: ~18 correct bits, one DVE op instead of two;
                # Z is in [e^-15, e^25], far from the undefined edge cases.
                st = state[q]
                if s == 0:
                    st["zi"] = zpool.tile([1, QW], F32, tag="zi",
                                          name=f"zi_{q}")
                nc.vector.reciprocal_approx_fast(
                    st["zi"][:, s * 512:(s + 1) * 512], st["pzt"][s][:])

            def bcast(q, s):
                st = state[q]
                if s == 0:
                    st["zb"] = zpool.tile([128, QW], F32, tag="zb",
                                          name=f"zb_{q}")
                nc.gpsimd.partition_broadcast(
                    st["zb"][:, s * 512:(s + 1) * 512],
                    st["zi"][:, s * 512:(s + 1) * 512])

            def mult(q, s):
                st = state[q]
                if s == 0:
                    st["ynt"] = ypool.tile([128, QW], F32R, tag="ynt",
                                           name=f"ynt_{q}")
                nc.vector.tensor_mul(
                    st["ynt"][:, s * 512:(s + 1) * 512],
                    st["pyt"][:, s * 512:(s + 1) * 512],
                    st["zb"][:, s * 512:(s + 1) * 512])

            def wproj(q, chunk, dma_eng=None):
                ob, s2 = divmod(chunk, 2)
                lo = q * QW + s2 * 512
                pwt = pw.tile([128, 512], F32, tag="pw",
                              name=f"pw_{q}_{chunk}")
                nc.tensor.matmul(
                    pwt[:], ww[:, ob * CI:(ob + 1) * CI],
                    state[q]["ynt"][:, s2 * 512:(s2 + 1) * 512],
                    start=True, stop=True)
                ot = opool.tile([128, 512], F32, tag="o", name=f"o_{q}_{chunk}")
                nc.vector.tensor_add(ot[:], pwt[:],
                                     xs[ob][:, lo:lo + 512])
                (dma_eng or nc.sync).dma_start(
                    out_d[ob * 128:(ob + 1) * 128, lo:lo + 512], ot[:])

            # Z accumulation on DVE (bf16 2x mode): binary-counter pairwise
            # tree for j < ZRUN_J, then in-place running adds, so only ONE
            # DVE op remains on the critical path after the last exp of the
            # quarter (a deep tree cascade there costs ~3.5us of tail).
            def tree_push(q, lvl, t_node):
                st = state[q]
                pend = st["pend"]
                if pend.get(lvl) is None:
                    pend[lvl] = t_node
                    return
                a, b = pend.pop(lvl), t_node
                out = ztpool.tile([128, QW], BF16, tag=f"l{lvl}",
                                  name=f"l{lvl}_{q}")
                nc.vector.tensor_add(out[:], a[:], b[:])
                tree_push(q, lvl + 1, out)

            def z_accum(q, j, ef):
                st = state[q]
                if j < ZRUN_J:
                    if j % 2 == 1:
                        pair = ztpool.tile([128, QW], BF16, tag="l0",
                                           name=f"l0_{q}_{j}")
                        nc.vector.tensor_add(pair[:], efs[q * JB + j - 1][:],
                                             ef[:])
                        tree_push(q, 1, pair)
                elif j == ZRUN_J:
                    # merge pending counter partials (sum of 0..23), then run
                    zq = st["zq"] = ztpool.tile([128, QW], BF16, tag="zq",
                                                name=f"zq_{q}")
                    p4, p3 = st["pend"].pop(4), st["pend"].pop(3)
                    nc.vector.tensor_add(zq[:], p4[:], p3[:])
                    nc.vector.tensor_add(zq[:], zq[:], ef[:])
                elif q < NQ - 1 or j < JB - 4:
                    nc.vector.tensor_add(st["zq"][:], st["zq"][:], ef[:])
                else:
                    # last 4 adds of the final quarter in halves: the s0 half
                    # of zq completes right after the last exp, so the tail's
                    # Z-close for s=0 (subtile dep) starts ~0.5us earlier
                    for s in range(2):
                        sl = slice(s * 512, (s + 1) * 512)
                        nc.vector.tensor_add(st["zq"][:, sl], st["zq"][:, sl],
                                             ef[:, sl])

            # ---------------- main flat pipeline ----------------
            for t in range(T + YLAG + 1):
                q, j = divmod(t, JB)
                if t < T:
                    if j == 0:
                        state[q] = {"pzt": [None, None], "pend": {}}
                    st = state[q]
                    i0 = q * QW
                    if t == 0:
                        pft = pf0          # s=0 half pre-emitted in preloop
                    else:
                        pft = pf.tile([128, QW], F32, tag="pf",
                                      name=f"pf_{t}")
                    for s in range(0 if t else 1, 2):
                        nc.tensor.matmul(
                            pft[:, s * 512:(s + 1) * 512],
                            ph_sb[:, j * 128:(j + 1) * 128],
                            th_sb[:, i0 + s * 512:i0 + (s + 1) * 512],
                            start=True, stop=True)
                    ef = efpool.tile([128, QW], BF16, tag="ef", name=f"ef_{t}")
                    efs[t] = ef
                    nc.scalar.activation(ef[:], pft[:], AF.Exp, bias=aux[:, 5:6])
                    z_accum(q, j, ef)
                    # previous quarter's deferred work
                    if q > 0:
                        if j == ZCLOSE_J[0]:
                            zclose(q - 1, 0)
                        elif j == ZCLOSE_J[1]:
                            zclose(q - 1, 1)
                        elif j == ZINV_J:
                            zinv(q - 1, 0)
                            zinv(q - 1, 1)
                        elif j == BCAST_J[0]:
                            bcast(q - 1, 0)
                        elif j == BCAST_J[1]:
                            bcast(q - 1, 1)
                        elif j == MULT_J:
                            mult(q - 1, 0)
                            mult(q - 1, 1)
                        elif WPROJ_J0 <= j < WPROJ_J0 + 4:
                            wproj(q - 1, j - WPROJ_J0)
                    for op in sched.get(t, []):
                        emit_sched_op(op)
                # trailing y accumulation
                ty = t - YLAG
                if 0 <= ty < T:
                    qy, jy = divmod(ty, JB)
                    if jy == 0:
                        state[qy]["pyt"] = py.tile([128, QW], F32, tag="py",
                                                   name=f"py_{qy}")
                    efy = efs.pop(ty)
                    for s in range(2):
                        nc.tensor.matmul(
                            state[qy]["pyt"][:, s * 512:(s + 1) * 512],
                            gT_sb[:, jy * 128:(jy + 1) * 128],
                            efy[:, s * 512:(s + 1) * 512],
                            start=(jy == 0), stop=(jy == JB - 1))

            # ---------------- last quarter's tail (pipelined by half) ------
            # Dummy matmuls bridge the PE-idle window while DVE/GpSimd run
            # the Z-finalize chain, so the HAM clock gate stays at 2.4 GHz
            # for the W-projection matmuls.
            def pe_dummy(n, i0):
                for i in range(n):
                    pd = pf.tile([128, 512], F32, tag="pf",
                                 name=f"dummy_{i0 + i}")
                    nc.tensor.matmul(pd[:], warm[:, 0:128], warm[:],
                                     start=True, stop=True)

            q = NQ - 1
            pe_dummy(2, 0)
            zclose(q, 0)
            zinv(q, 0)
            bcast(q, 0)
            zclose(q, 1)
            zinv(q, 1)
            bcast(q, 1)
            pe_dummy(12, 2)
            mult(q, 0)
            wproj(q, 0, nc.scalar)   # both s0 chunks before mult(s1);
            wproj(q, 2)              # out-DMAs alternate the two queues
            mult(q, 1)
            wproj(q, 1, nc.scalar)
            wproj(q, 3)

    nc.compile()
    return nc


_CACHE = {}


def _get_nc():
    if "nc" not in _CACHE:
        _CACHE["nc"] = build()
    return _CACHE["nc"]


def _in_maps(x, g_w, g_b, theta_w, theta_b, phi_w, phi_b, W_w, W_b):
    x = np.ascontiguousarray(np.asarray(x, dtype=np.float32))
    wbe = (np.asarray(W_w, np.float32) @ np.asarray(g_b, np.float32)
           + np.asarray(W_b, np.float32))
    common = {
        "thw_t": np.ascontiguousarray(np.asarray(theta_w, np.float32).T),
        "phw_t": np.ascontiguousarray(np.asarray(phi_w, np.float32).T),
        "gw_t": np.ascontiguousarray(np.asarray(g_w, np.float32).T),
        "ww_t": np.ascontiguousarray(np.asarray(W_w, np.float32).T),
        "aux": np.stack(
            [
                np.asarray(theta_b, np.float32),
                np.asarray(phi_b, np.float32),
                wbe[:128],
                wbe[128:],
                np.ones(128, np.float32),
                np.full(128, -40.0, np.float32),
            ],
            axis=1,
        ),
    }
    return [
        {"x": np.ascontiguousarray(x[b].reshape(C, N)), **common}
        for b in range(B)
    ]


def run(in_maps, **kw):
    nc = _get_nc()
    return run_bass_kernel_spmd(nc, in_maps, list(range(B)), **kw)


def kernel(**inputs):
    res = run(_in_maps(**inputs))
    out = np.stack([res.results[b]["out"] for b in range(B)])
    return out.reshape(B, C, H, Wd)


# revision 47
# speedup vs baseline: 1.1803x; 1.1803x over previous
"""NonLocalBlock (embedded-gaussian attention) TRN2 kernel, v2.

Shapes (hardcoded): x [8, 256, 64, 64] fp32, one batch element per core.
Per core:
  theta/phi/g = 1x1 conv projections of x_b [256, 4096] -> [128, 4096]
  f^T[j, i] = sum_c phi[c, j] theta[c, i]        (4096 x 4096 logits)
  ef = exp(f - 40) in bf16 (shift cancels in y/Z; avoids bf16 overflow)
  y[ci, i] = sum_j ef[j, i] gT[j, ci]            (bf16 matmuls, fp32 PSUM)
  Z[i] = sum_j ef[j, i]  via bf16 adds on DVE (binary-counter tree for
         j<24, then in-place running adds so only one DVE op trails the
         last exp), closed by a ones-matmul partition reduce on PE
  out = x + W_w @ (y / Z) + (W_w @ g_b + W_b)    (g bias folded, sum soft=1)

Steady state is ScalarE-bound (exp of 16.7M elems at ~1117 ns per 1024-col
tile; measured main loop = 128 x 1117 ns, Scalar ~100% busy).  Everything
else is kept under that cadence:
  - PE per t: f 2x512-col f32r + y 2x512-col bf16 ~ 930 ns; Z is NOT on PE
    (except 2 closing 512-col ones-MMs per quarter).
  - DVE per t: Z adds (bf16 2x mode, ~690 ns each) + piece evictions.
  - GpSimd only does the 1/Z partition broadcasts (its slow tensor ops
    share the DVE SBUF port and starve 2-port DVE instructions).
  - x is DMAed in column chunks on both hw queues (the rings ramp slowly:
    ~60 GB/s cold, so the first 512-col chunk is split out to land early);
    projections run as 512-col pieces: chunks 0/1 pre-loop, the rest
    injected into main-loop PE slack with a deadline-based schedule
    (in-loop PSUM pieces share the pw tag with W-proj/Z-close tiles).
  - Warmup matmuls (plus a few interleaved between pre-loop pieces) and a
    dummy activation bridge the DMA wait so the HAM clock gate never drops
    to 1.2 GHz and the exp table load is off the critical path.
  - Per-quarter deferred ops in the next quarter: Z-close j=3,4; recip
    (fast, ~18 bits) j=5; broadcast j=6,7; normalize j=11 (= YLAG, first
    slot after the y accumulation group closes); W-projection j=12..15.
  - Tail: dummy matmuls keep the PE clock warm through the Z-finalize
    chain; final out-DMAs alternate both queues.
"""

import numpy as np

import concourse.bacc as bacc
import concourse.mybir as mybir
from concourse import tile
from concourse.bass_utils import run_bass_kernel_spmd

F32 = mybir.dt.float32
F32R = mybir.dt.float32r
BF16 = mybir.dt.bfloat16
AF = mybir.ActivationFunctionType

B, C, CI = 8, 256, 128
H, Wd = 64, 64
N = H * Wd              # 4096
NQ = 4                  # i-quarters
QW = N // NQ            # 1024
JB = N // 128           # 32 j-blocks (= ts per quarter)
T = NQ * JB             # 128
NCH = 8                 # 512-col x pieces for projections
PW = N // NCH           # 512

YLAG = 11               # y-matmul lag behind f/exp
ZCLOSE_J = (3, 4)       # prev quarter's Z partition-reduce MMs
ZINV_J = 5              # prev quarter's reciprocals
BCAST_J = (6, 7)        # prev quarter's Z broadcast (gpsimd)
MULT_J = 11             # prev quarter's normalize (= YLAG: y-accum group
                        # closes at j=YLAG-1, so this is the first safe slot)
WPROJ_J0 = 12           # prev quarter's W-projection chunks (4)
EXP_BIAS = -40.0
NWARM = 11              # warmup matmuls: bridge PE activity from the start
                        # barrier (~7.4us) to x-chunk-A1 landing (~11.3us) so
                        # the HAM clock gate never drops to 1.2 GHz
ZRUN_J = 24             # switch Z accumulation from tree to running adds here


def _build_sched():
    # t -> list of ops. pieces are 512-col units c=0..7 of x columns.
    # deadlines: ph piece c before f uses j-block 4c (t=4c); th piece c
    # before f of quarter c//2 (t=32*(c//2)); g piece c before y uses
    # block 4c (t=YLAG+4c).
    sched = {}

    def add(t, op):
        sched.setdefault(t, []).append(op)

    add(1, ("ph", 1))
    for c in range(2, 8):
        add(4 * c - 3, ("ph", c))
    for c, t in ((2, 24), (3, 26), (4, 48), (5, 52), (6, 80), (7, 84)):
        add(t, ("th", c))
    gsched = {0: 2, 1: 6, 2: 10, 3: 14, 4: 18, 5: 22, 6: 30, 7: 33}
    for c, t in gsched.items():
        add(t, ("ga", c))
        add(t + 1, ("gb", c))
    # bias add for chunk c after last of {th_c, ph_c, gb_c}
    for c, t in ((0, 4), (1, 8), (2, 25), (3, 27), (4, 49), (5, 53),
                 (6, 81), (7, 85)):
        add(t, ("bias", c))
    return sched


def build():
    nc = bacc.Bacc("TRN2", target_bir_lowering=False, debug=False, num_devices=8)

    x_d = nc.dram_tensor("x", [C, N], F32R, kind="ExternalInput")
    thw_d = nc.dram_tensor("thw_t", [C, CI], F32R, kind="ExternalInput")  # theta_w.T
    phw_d = nc.dram_tensor("phw_t", [C, CI], F32R, kind="ExternalInput")  # phi_w.T
    gw_d = nc.dram_tensor("gw_t", [C, CI], F32R, kind="ExternalInput")    # g_w.T
    ww_d = nc.dram_tensor("ww_t", [CI, C], F32R, kind="ExternalInput")    # W_w.T
    # aux cols: 0=theta_b, 1=phi_b, 2=wb_eff[:128], 3=wb_eff[128:], 4=ones,
    # 5=exp bias (-40)
    aux_d = nc.dram_tensor("aux", [128, 6], F32, kind="ExternalInput")
    out_d = nc.dram_tensor("out", [C, N], F32, kind="ExternalOutput")

    sched = _build_sched()

    with tile.TileContext(nc) as tc:
        with (
            tc.tile_pool(name="const", bufs=1) as cpool,
            tc.tile_pool(name="big", bufs=1) as bigpool,
            tc.tile_pool(name="ef", bufs=13) as efpool,
            tc.tile_pool(name="ztree", bufs=2) as ztpool,
            tc.tile_pool(name="zpool", bufs=2) as zpool,
            tc.tile_pool(name="ypool", bufs=2) as ypool,
            tc.tile_pool(name="opool", bufs=6) as opool,
            tc.tile_pool(name="pf", bufs=2, space="PSUM") as pf,
            tc.tile_pool(name="py", bufs=1, space="PSUM") as py,
            tc.tile_pool(name="pw", bufs=2, space="PSUM") as pw,
        ):
            # ---------------- warmup + DMA issue ----------------
            warm = cpool.tile([128, 512], BF16, tag="warm")
            warm2 = cpool.tile([128, 1], F32, tag="warm2")
            nc.gpsimd.memset(warm[:], 0.0)

            aux = cpool.tile([128, 6], F32, tag="aux")
            thw = cpool.tile([128, 2 * CI], F32R, tag="thw")
            phw = cpool.tile([128, 2 * CI], F32R, tag="phw")
            gw = cpool.tile([128, 2 * CI], F32R, tag="gw")
            ww = cpool.tile([CI, C], F32R, tag="ww")
            x0 = bigpool.tile([128, N], F32R, tag="x0")
            x1 = bigpool.tile([128, N], F32R, tag="x1")
            xs = (x0, x1)

            # x chunk A first on both queues (the DMA rings are slow to ramp;
            # chunk A gates the first projections), then weights.  The dummy
            # activation pulls the ~2.7us exp table load off the critical
            # path, but only after the two most urgent scalar-queue issues.
            # first x chunk split in two so the 0:512 half (which gates the
            # first projection pieces) lands ~4us earlier on the cold rings
            nc.sync.dma_start(x0[:, 0:512], x_d[0:128, 0:512])
            nc.scalar.dma_start(x1[:, 0:512], x_d[128:256, 0:512])
            nc.sync.dma_start(thw[:, 0:CI], thw_d[0:128, :])
            nc.scalar.dma_start(thw[:, CI:2 * CI], thw_d[128:256, :])
            nc.scalar.activation(warm2[:], warm[:, 0:1], AF.Identity)
            nc.sync.dma_start(aux[:], aux_d[:])
            nc.sync.dma_start(phw[:, 0:CI], phw_d[0:128, :])
            nc.scalar.dma_start(phw[:, CI:2 * CI], phw_d[128:256, :])
            nc.sync.dma_start(gw[:, 0:CI], gw_d[0:128, :])
            nc.scalar.dma_start(gw[:, CI:2 * CI], gw_d[128:256, :])
            nc.sync.dma_start(x0[:, 512:1024], x_d[0:128, 512:1024])
            nc.scalar.dma_start(x1[:, 512:1024], x_d[128:256, 512:1024])

            for _ in range(NWARM):
                pwt = pw.tile([128, 512], F32, tag="pw", name="warm_mm")
                nc.tensor.matmul(pwt[:], warm[:, 0:128], warm[:],
                                 start=True, stop=True)

            thb, phb = aux[:, 0:1], aux[:, 1:2]
            wbe = (aux[:, 2:3], aux[:, 3:4])
            ones_bf = cpool.tile([128, 1], BF16, tag="ones_bf")
            nc.vector.tensor_copy(ones_bf[:], aux[:, 4:5])

            th_sb = bigpool.tile([128, N], F32R, tag="th")
            ph_sb = bigpool.tile([128, N], F32R, tag="ph")
            gT_sb = bigpool.tile([128, N], BF16, tag="gT")

            # ---------------- projection piece emitters ----------------
            def proj_piece(kind, c, ev="v", pool_tag="pw"):
                # kind in ("th", "ph"): [ci, 512] piece of theta/phi.
                # ev: "v" DVE eviction (in-loop: ScalarE is the exp
                # bottleneck), "s" ScalarE (pre-loop parallelism).
                # pool_tag "pf" pre-loop avoids pw-rotation serialization.
                wt, bias_t, dst = ((thw, thb, th_sb) if kind == "th"
                                   else (phw, phb, ph_sb))
                lo = c * PW
                pool = pf if pool_tag == "pf" else pw
                pp = pool.tile([128, 512], F32, tag=pool_tag,
                               name=f"{kind}p_{c}")
                for k in range(2):
                    nc.tensor.matmul(
                        pp[:], wt[:, k * CI:(k + 1) * CI],
                        xs[k][:, lo:lo + 512],
                        start=(k == 0), stop=(k == 1),
                    )
                if ev == "s":
                    nc.scalar.activation(dst[:, lo:lo + 512], pp[:],
                                         AF.Identity, bias=bias_t)
                else:
                    nc.vector.tensor_scalar_add(dst[:, lo:lo + 512], pp[:],
                                                bias_t)

            gtiles = {}

            def g_piece(c, half):
                # gT blocks 4c+2*half, 4c+2*half+1 into shared [128,512] tile
                if half == 0:
                    gtiles[c] = pw.tile([128, 512], F32, tag="pw",
                                        name=f"gp_{c}")
                pg = gtiles[c]
                for b in (4 * c + 2 * half, 4 * c + 2 * half + 1):
                    col = (b - 4 * c) * 128
                    for k in range(2):
                        nc.tensor.matmul(
                            pg[:, col:col + 128],
                            xs[k][:, b * 128:(b + 1) * 128],
                            gw[:, k * CI:(k + 1) * CI],
                            start=(k == 0), stop=(k == 1),
                        )
                if half == 1:
                    lo = c * PW
                    nc.vector.tensor_copy(gT_sb[:, lo:lo + 512], pg[:])
                    del gtiles[c]

            def bias_chunk(c):
                # DVE, not GpSimd: GpSimd shares its SBUF port with the DVE
                # and its slow tensor ops starve 2-port DVE instructions.
                lo = c * PW
                for k in range(2):
                    nc.vector.tensor_scalar_add(
                        xs[k][:, lo:lo + 512], xs[k][:, lo:lo + 512], wbe[k])

            def emit_sched_op(op):
                kind = op[0]
                if kind in ("th", "ph"):
                    proj_piece(kind, op[1])
                elif kind == "ga":
                    g_piece(op[1], 0)
                elif kind == "gb":
                    g_piece(op[1], 1)
                elif kind == "bias":
                    bias_chunk(op[1])

            # preloop: th chunks 0,1 + ph chunk 0.  A1-gated pieces first
            # (th0, ph0), A2-gated th1 last; evictions split scalar/DVE so
            # they pipeline.  Warmup matmuls interleave into the dead PE
            # windows (waiting for x-A2 / evictions) so the HAM activity
            # monitor never sees a ~3.4us idle window and re-throttles.
            def pre_warm(n):
                for _ in range(n):
                    pwt = pw.tile([128, 512], F32, tag="pw", name="warm_mm")
                    nc.tensor.matmul(pwt[:], warm[:, 0:128], warm[:],
                                     start=True, stop=True)

            proj_piece("th", 0, ev="s")
            proj_piece("ph", 0, ev="v")
            pre_warm(7)
            proj_piece("th", 1, ev="s")
            pre_warm(3)

            # deferred x DMA chunks
            nc.sync.dma_start(x0[:, 1024:2048], x_d[0:128, 1024:2048])
            nc.scalar.dma_start(x1[:, 1024:2048], x_d[128:256, 1024:2048])
            nc.sync.dma_start(x0[:, 2048:4096], x_d[0:128, 2048:4096])
            nc.scalar.dma_start(x1[:, 2048:4096], x_d[128:256, 2048:4096])
            nc.sync.dma_start(ww[:], ww_d[:])

            # ---------------- per-quarter deferred ops ----------------
            state = {}   # per-quarter: zq, pzt[2], zi, zb, pyt, ynt
            efs = {}

            def zclose(q, s):
                st = state[q]
                pzt = pw.tile([1, 512], F32, tag="pw", name=f"pz_{q}_{s}")
                st["pzt"][s] = pzt
                nc.tensor.matmul(pzt[:], ones_bf[:],
                                 st["zq"][:, s * 512:(s + 1) * 512],
                                 start=True, stop=True)

            def zinv(q, s):
                # fast variant: ~18 correct bits, one DVE op instead of two;
                # Z is in [e^-15, e^25], far from the undefined edge cases.
                st = state[q]
                if s == 0:
                    st["zi"] = zpool.tile([1, QW], F32, tag="zi",
                                          name=f"zi_{q}")
                nc.vector.reciprocal_approx_fast(
                    st["zi"][:, s * 512:(s + 1) * 512], st["pzt"][s][:])

            def bcast(q, s):
                st = state[q]
                if s == 0:
                    st["zb"] = zpool.tile([128, QW], F32, tag="zb",
                                          name=f"zb_{q}")
                nc.gpsimd.partition_broadcast(
                    st["zb"][:, s * 512:(s + 1) * 512],
                    st["zi"][:, s * 512:(s + 1) * 512])

            def mult(q, s):
                st = state[q]
                if s == 0:
                    st["ynt"] = ypool.tile([128, QW], F32R, tag="ynt",
                                           name=f"ynt_{q}")
                nc.vector.tensor_mul(
                    st["ynt"][:, s * 512:(s + 1) * 512],
                    st["pyt"][:, s * 512:(s + 1) * 512],
                    st["zb"][:, s * 512:(s + 1) * 512])

            def wproj(q, chunk, dma_eng=None):
                ob, s2 = divmod(chunk, 2)
                lo = q * QW + s2 * 512
                pwt = pw.tile([128, 512], F32, tag="pw",
                              name=f"pw_{q}_{chunk}")
                nc.tensor.matmul(
                    pwt[:], ww[:, ob * CI:(ob + 1) * CI],
                    state[q]["ynt"][:, s2 * 512:(s2 + 1) * 512],
                    start=True, stop=True)
                ot = opool.tile([128, 512], F32, tag="o", name=f"o_{q}_{chunk}")
                nc.vector.tensor_add(ot[:], pwt[:],
                                     xs[ob][:, lo:lo + 512])
                (dma_eng or nc.sync).dma_start(
                    out_d[ob * 128:(ob + 1) * 128, lo:lo + 512], ot[:])

            # Z accumulation on DVE (bf16 2x mode): binary-counter pairwise
            # tree for j < ZRUN_J, then in-place running adds, so only ONE
            # DVE op remains on the critical path after the last exp of the
            # quarter (a deep tree cascade there costs ~3.5us of tail).
            def tree_push(q, lvl, t_node):
                st = state[q]
                pend = st["pend"]
                if pend.get(lvl) is None:
                    pend[lvl] = t_node
                    return
                a, b = pend.pop(lvl), t_node
                out = ztpool.tile([128, QW], BF16, tag=f"l{lvl}",
                                  name=f"l{lvl}_{q}")
                nc.vector.tensor_add(out[:], a[:], b[:])
                tree_push(q, lvl + 1, out)

            def z_accum(q, j, ef):
                st = state[q]
                if j < ZRUN_J:
                    if j % 2 == 1:
                        pair = ztpool.tile([128, QW], BF16, tag="l0",
                                           name=f"l0_{q}_{j}")
                        nc.vector.tensor_add(pair[:], efs[q * JB + j - 1][:],
                                             ef[:])
                        tree_push(q, 1, pair)
                elif j == ZRUN_J:
                    # merge pending counter partials (sum of 0..23), then run
                    zq = st["zq"] = ztpool.tile([128, QW], BF16, tag="zq",
                                                name=f"zq_{q}")
                    p4, p3 = st["pend"].pop(4), st["pend"].pop(3)
                    nc.vector.tensor_add(zq[:], p4[:], p3[:])
                    nc.vector.tensor_add(zq[:], zq[:], ef[:])
                elif q < NQ - 1 or j < JB - 4:
                    nc.vector.tensor_add(st["zq"][:], st["zq"][:], ef[:])
                else:
                    # last 4 adds of the final quarter in halves: the s0 half
                    # of zq completes right after the last exp, so the tail's
                    # Z-close for s=0 (subtile dep) starts ~0.5us earlier
                    for s in range(2):
                        sl = slice(s * 512, (s + 1) * 512)
                        nc.vector.tensor_add(st["zq"][:, sl], st["zq"][:, sl],
                                             ef[:, sl])

            # ---------------- main flat pipeline ----------------
            for t in range(T + YLAG + 1):
                q, j = divmod(t, JB)
                if t < T:
                    if j == 0:
                        state[q] = {"pzt": [None, None], "pend": {}}
                    st = state[q]
                    i0 = q * QW
                    pft = pf.tile([128, QW], F32, tag="pf", name=f"pf_{t}")
                    for s in range(2):
                        nc.tensor.matmul(
                            pft[:, s * 512:(s + 1) * 512],
                            ph_sb[:, j * 128:(j + 1) * 128],
                            th_sb[:, i0 + s * 512:i0 + (s + 1) * 512],
                            start=True, stop=True)
                    ef = efpool.tile([128, QW], BF16, tag="ef", name=f"ef_{t}")
                    efs[t] = ef
                    nc.scalar.activation(ef[:], pft[:], AF.Exp, bias=aux[:, 5:6])
                    z_accum(q, j, ef)
                    # previous quarter's deferred work
                    if q > 0:
                        if j == ZCLOSE_J[0]:
                            zclose(q - 1, 0)
                        elif j == ZCLOSE_J[1]:
                            zclose(q - 1, 1)
                        elif j == ZINV_J:
                            zinv(q - 1, 0)
                            zinv(q - 1, 1)
                        elif j == BCAST_J[0]:
                            bcast(q - 1, 0)
                        elif j == BCAST_J[1]:
                            bcast(q - 1, 1)
                        elif j == MULT_J:
                            mult(q - 1, 0)
                            mult(q - 1, 1)
                        elif WPROJ_J0 <= j < WPROJ_J0 + 4:
                            wproj(q - 1, j - WPROJ_J0)
                    for op in sched.get(t, []):
                        emit_sched_op(op)
                # trailing y accumulation
                ty = t - YLAG
                if 0 <= ty < T:
                    qy, jy = divmod(ty, JB)
                    if jy == 0:
                        state[qy]["pyt"] = py.tile([128, QW], F32, tag="py",
                                                   name=f"py_{qy}")
                    efy = efs.pop(ty)
                    for s in range(2):
                        nc.tensor.matmul(
                            state[qy]["pyt"][:, s * 512:(s + 1) * 512],
                            gT_sb[:, jy * 128:(jy + 1) * 128],
                            efy[:, s * 512:(s + 1) * 512],
                            start=(jy == 0), stop=(jy == JB - 1))

            # ---------------- last quarter's tail (pipelined by half) ------
            # Dummy matmuls bridge the PE-idle window while DVE/GpSimd run
            # the Z-finalize chain, so the HAM clock gate stays at 2.4 GHz
            # for the W-projection matmuls.
            def pe_dummy(n, i0):
                for i in range(n):
                    pd = pf.tile([128, 512], F32, tag="pf",
                                 name=f"dummy_{i0 + i}")
                    nc.tensor.matmul(pd[:], warm[:, 0:128], warm[:],
                                     start=True, stop=True)

            q = NQ - 1
            pe_dummy(2, 0)
            zclose(q, 0)
            zinv(q, 0)
            bcast(q, 0)
            zclose(q, 1)
            zinv(q, 1)
            bcast(q, 1)
            pe_dummy(12, 2)
            mult(q, 0)
            wproj(q, 0, nc.scalar)   # both s0 chunks before mult(s1);
            wproj(q, 2)              # out-DMAs alternate the two queues
            mult(q, 1)
            wproj(q, 1, nc.scalar)
            wproj(q, 3)

    nc.compile()
    return nc


_CACHE = {}


def _get_nc():
    if "nc" not in _CACHE:
        _CACHE["nc"] = build()
    return _CACHE["nc"]


def _in_maps(x, g_w, g_b, theta_w, theta_b, phi_w, phi_b, W_w, W_b):
    x = np.ascontiguousarray(np.asarray(x, dtype=np.float32))
    wbe = (np.asarray(W_w, np.float32) @ np.asarray(g_b, np.float32)
           + np.asarray(W_b, np.float32))
    common = {
        "thw_t": np.ascontiguousarray(np.asarray(theta_w, np.float32).T),
        "phw_t": np.ascontiguousarray(np.asarray(phi_w, np.float32).T),
        "gw_t": np.ascontiguousarray(np.asarray(g_w, np.float32).T),
        "ww_t": np.ascontiguousarray(np.asarray(W_w, np.float32).T),
        "aux": np.stack(
            [
                np.asarray(theta_b, np.float32),
                np.asarray(phi_b, np.float32),
                wbe[:128],
                wbe[128:],
                np.ones(128, np.float32),
                np.full(128, -40.0, np.float32),
            ],
            axis=1,
        ),
    }
    return [
        {"x": np.ascontiguousarray(x[b].reshape(C, N)), **common}
        for b in range(B)
    ]


def run(in_maps, **kw):
    nc = _get_nc()
    return run_bass_kernel_spmd(nc, in_maps, list(range(B)), **kw)


def kernel(**inputs):
    res = run(_in_maps(**inputs))
    out = np.stack([res.results[b]["out"] for b in range(B)])
    return out.reshape(B, C, H, Wd)


# revision 48
# speedup vs baseline: 1.1911x; 1.0092x over previous
"""NonLocalBlock (embedded-gaussian attention) TRN2 kernel, v2.

Shapes (hardcoded): x [8, 256, 64, 64] fp32, one batch element per core.
Per core:
  theta/phi/g = 1x1 conv projections of x_b [256, 4096] -> [128, 4096]
  f^T[j, i] = sum_c phi[c, j] theta[c, i]        (4096 x 4096 logits)
  ef = exp(f - 40) in bf16 (shift cancels in y/Z; avoids bf16 overflow)
  y[ci, i] = sum_j ef[j, i] gT[j, ci]            (bf16 matmuls, fp32 PSUM)
  Z[i] = sum_j ef[j, i]  via bf16 adds on DVE (binary-counter tree for
         j<24, then in-place running adds so only one DVE op trails the
         last exp), closed by a ones-matmul partition reduce on PE
  out = x + W_w @ (y / Z) + (W_w @ g_b + W_b)    (g bias folded, sum soft=1)

Steady state is ScalarE-bound (exp of 16.7M elems at ~1117 ns per 1024-col
tile; measured main loop = 128 x 1117 ns, Scalar ~100% busy).  Everything
else is kept under that cadence:
  - PE per t: f 2x512-col f32r + y 2x512-col bf16 ~ 930 ns; Z is NOT on PE
    (except 2 closing 512-col ones-MMs per quarter).
  - DVE per t: Z adds (bf16 2x mode, ~690 ns each) + piece evictions.
  - GpSimd only does the 1/Z partition broadcasts (its slow tensor ops
    share the DVE SBUF port and starve 2-port DVE instructions).
  - x is DMAed in column chunks on both hw queues (the rings ramp slowly:
    ~60 GB/s cold, so the first 512-col chunk is split out to land early);
    projections run as 512-col pieces: chunks 0/1 pre-loop, the rest
    injected into main-loop PE slack with a deadline-based schedule
    (in-loop PSUM pieces share the pw tag with W-proj/Z-close tiles).
  - Warmup matmuls (plus a few interleaved between pre-loop pieces) and a
    dummy activation bridge the DMA wait so the HAM clock gate never drops
    to 1.2 GHz and the exp table load is off the critical path.
  - Per-quarter deferred ops in the next quarter: Z-close j=3,4; recip
    (fast, ~18 bits) j=5; broadcast j=6,7; normalize j=11 (= YLAG, first
    slot after the y accumulation group closes); W-projection j=12..15.
  - Tail: dummy matmuls keep the PE clock warm through the Z-finalize
    chain; final out-DMAs alternate both queues.
"""

import numpy as np

import concourse.bacc as bacc
import concourse.mybir as mybir
from concourse import tile
from concourse.bass_utils import run_bass_kernel_spmd

F32 = mybir.dt.float32
F32R = mybir.dt.float32r
BF16 = mybir.dt.bfloat16
AF = mybir.ActivationFunctionType

B, C, CI = 8, 256, 128
H, Wd = 64, 64
N = H * Wd              # 4096
NQ = 4                  # i-quarters
QW = N // NQ            # 1024
JB = N // 128           # 32 j-blocks (= ts per quarter)
T = NQ * JB             # 128
NCH = 8                 # 512-col x pieces for projections
PW = N // NCH           # 512

YLAG = 11               # y-matmul lag behind f/exp
ZCLOSE_J = (3, 4)       # prev quarter's Z partition-reduce MMs
ZINV_J = 5              # prev quarter's reciprocals
BCAST_J = (6, 7)        # prev quarter's Z broadcast (gpsimd)
MULT_J = 11             # prev quarter's normalize (= YLAG: y-accum group
                        # closes at j=YLAG-1, so this is the first safe slot)
WPROJ_J0 = 12           # prev quarter's W-projection chunks (4)
EXP_BIAS = -40.0
NWARM = 11              # warmup matmuls: bridge PE activity from the start
                        # barrier (~7.4us) to x-chunk-A1 landing (~11.3us) so
                        # the HAM clock gate never drops to 1.2 GHz
ZRUN_J = 24             # switch Z accumulation from tree to running adds here


def _build_sched():
    # t -> list of ops. pieces are 512-col units c=0..7 of x columns.
    # deadlines: ph piece c before f uses j-block 4c (t=4c); th piece c
    # before f of quarter c//2 (t=32*(c//2)); g piece c before y uses
    # block 4c (t=YLAG+4c).
    sched = {}

    def add(t, op):
        sched.setdefault(t, []).append(op)

    add(1, ("ph", 1))
    for c in range(2, 8):
        add(4 * c - 3, ("ph", c))
    for c, t in ((2, 24), (3, 26), (4, 48), (5, 52), (6, 80), (7, 84)):
        add(t, ("th", c))
    gsched = {0: 2, 1: 6, 2: 10, 3: 14, 4: 18, 5: 22, 6: 30, 7: 33}
    for c, t in gsched.items():
        add(t, ("ga", c))
        add(t + 1, ("gb", c))
    # bias add for chunk c after last of {th_c, ph_c, gb_c}
    for c, t in ((0, 4), (1, 8), (2, 25), (3, 27), (4, 49), (5, 53),
                 (6, 81), (7, 85)):
        add(t, ("bias", c))
    return sched


def build():
    nc = bacc.Bacc("TRN2", target_bir_lowering=False, debug=False, num_devices=8)

    x_d = nc.dram_tensor("x", [C, N], F32R, kind="ExternalInput")
    thw_d = nc.dram_tensor("thw_t", [C, CI], F32R, kind="ExternalInput")  # theta_w.T
    phw_d = nc.dram_tensor("phw_t", [C, CI], F32R, kind="ExternalInput")  # phi_w.T
    gw_d = nc.dram_tensor("gw_t", [C, CI], F32R, kind="ExternalInput")    # g_w.T
    ww_d = nc.dram_tensor("ww_t", [CI, C], F32R, kind="ExternalInput")    # W_w.T
    # aux cols: 0=theta_b, 1=phi_b, 2=wb_eff[:128], 3=wb_eff[128:], 4=ones,
    # 5=exp bias (-40)
    aux_d = nc.dram_tensor("aux", [128, 6], F32, kind="ExternalInput")
    out_d = nc.dram_tensor("out", [C, N], F32, kind="ExternalOutput")

    sched = _build_sched()

    with tile.TileContext(nc) as tc:
        with (
            tc.tile_pool(name="const", bufs=1) as cpool,
            tc.tile_pool(name="big", bufs=1) as bigpool,
            tc.tile_pool(name="ef", bufs=13) as efpool,
            tc.tile_pool(name="ztree", bufs=2) as ztpool,
            tc.tile_pool(name="zpool", bufs=2) as zpool,
            tc.tile_pool(name="ypool", bufs=2) as ypool,
            tc.tile_pool(name="opool", bufs=6) as opool,
            tc.tile_pool(name="pf", bufs=2, space="PSUM") as pf,
            tc.tile_pool(name="py", bufs=1, space="PSUM") as py,
            tc.tile_pool(name="pw", bufs=2, space="PSUM") as pw,
        ):
            # ---------------- warmup + DMA issue ----------------
            warm = cpool.tile([128, 512], BF16, tag="warm")
            warm2 = cpool.tile([128, 1], F32, tag="warm2")
            nc.gpsimd.memset(warm[:], 0.0)

            aux = cpool.tile([128, 6], F32, tag="aux")
            thw = cpool.tile([128, 2 * CI], F32R, tag="thw")
            phw = cpool.tile([128, 2 * CI], F32R, tag="phw")
            gw = cpool.tile([128, 2 * CI], F32R, tag="gw")
            ww = cpool.tile([CI, C], F32R, tag="ww")
            x0 = bigpool.tile([128, N], F32R, tag="x0")
            x1 = bigpool.tile([128, N], F32R, tag="x1")
            xs = (x0, x1)

            # x chunk A first on both queues (the DMA rings are slow to ramp;
            # chunk A gates the first projections), then weights.  The dummy
            # activation pulls the ~2.7us exp table load off the critical
            # path, but only after the two most urgent scalar-queue issues.
            # first x chunk split in two so the 0:512 half (which gates the
            # first projection pieces) lands ~4us earlier on the cold rings
            nc.sync.dma_start(x0[:, 0:512], x_d[0:128, 0:512])
            nc.scalar.dma_start(x1[:, 0:512], x_d[128:256, 0:512])
            nc.sync.dma_start(thw[:, 0:CI], thw_d[0:128, :])
            nc.scalar.dma_start(thw[:, CI:2 * CI], thw_d[128:256, :])
            nc.scalar.activation(warm2[:], warm[:, 0:1], AF.Identity)
            nc.sync.dma_start(aux[:], aux_d[:])
            nc.sync.dma_start(phw[:, 0:CI], phw_d[0:128, :])
            nc.scalar.dma_start(phw[:, CI:2 * CI], phw_d[128:256, :])
            nc.sync.dma_start(gw[:, 0:CI], gw_d[0:128, :])
            nc.scalar.dma_start(gw[:, CI:2 * CI], gw_d[128:256, :])
            nc.sync.dma_start(x0[:, 512:1024], x_d[0:128, 512:1024])
            nc.scalar.dma_start(x1[:, 512:1024], x_d[128:256, 512:1024])

            for _ in range(NWARM):
                pwt = pw.tile([128, 512], F32, tag="pw", name="warm_mm")
                nc.tensor.matmul(pwt[:], warm[:, 0:128], warm[:],
                                 start=True, stop=True)

            thb, phb = aux[:, 0:1], aux[:, 1:2]
            wbe = (aux[:, 2:3], aux[:, 3:4])
            ones_bf = cpool.tile([128, 1], BF16, tag="ones_bf")
            nc.vector.tensor_copy(ones_bf[:], aux[:, 4:5])

            th_sb = bigpool.tile([128, N], F32R, tag="th")
            ph_sb = bigpool.tile([128, N], F32R, tag="ph")
            gT_sb = bigpool.tile([128, N], BF16, tag="gT")

            # ---------------- projection piece emitters ----------------
            def proj_piece(kind, c, ev="v", pool_tag="pw"):
                # kind in ("th", "ph"): [ci, 512] piece of theta/phi.
                # ev: "v" DVE eviction (in-loop: ScalarE is the exp
                # bottleneck), "s" ScalarE (pre-loop parallelism).
                # pool_tag "pf" pre-loop avoids pw-rotation serialization.
                wt, bias_t, dst = ((thw, thb, th_sb) if kind == "th"
                                   else (phw, phb, ph_sb))
                lo = c * PW
                pool = pf if pool_tag == "pf" else pw
                pp = pool.tile([128, 512], F32, tag=pool_tag,
                               name=f"{kind}p_{c}")
                for k in range(2):
                    nc.tensor.matmul(
                        pp[:], wt[:, k * CI:(k + 1) * CI],
                        xs[k][:, lo:lo + 512],
                        start=(k == 0), stop=(k == 1),
                    )
                if ev == "s":
                    nc.scalar.activation(dst[:, lo:lo + 512], pp[:],
                                         AF.Identity, bias=bias_t)
                else:
                    nc.vector.tensor_scalar_add(dst[:, lo:lo + 512], pp[:],
                                                bias_t)

            gtiles = {}

            def g_piece(c, half):
                # gT blocks 4c+2*half, 4c+2*half+1 into shared [128,512] tile
                if half == 0:
                    gtiles[c] = pw.tile([128, 512], F32, tag="pw",
                                        name=f"gp_{c}")
                pg = gtiles[c]
                for b in (4 * c + 2 * half, 4 * c + 2 * half + 1):
                    col = (b - 4 * c) * 128
                    for k in range(2):
                        nc.tensor.matmul(
                            pg[:, col:col + 128],
                            xs[k][:, b * 128:(b + 1) * 128],
                            gw[:, k * CI:(k + 1) * CI],
                            start=(k == 0), stop=(k == 1),
                        )
                if half == 1:
                    lo = c * PW
                    nc.vector.tensor_copy(gT_sb[:, lo:lo + 512], pg[:])
                    del gtiles[c]

            def bias_chunk(c):
                # DVE, not GpSimd: GpSimd shares its SBUF port with the DVE
                # and its slow tensor ops starve 2-port DVE instructions.
                lo = c * PW
                for k in range(2):
                    nc.vector.tensor_scalar_add(
                        xs[k][:, lo:lo + 512], xs[k][:, lo:lo + 512], wbe[k])

            def emit_sched_op(op):
                kind = op[0]
                if kind in ("th", "ph"):
                    proj_piece(kind, op[1])
                elif kind == "ga":
                    g_piece(op[1], 0)
                elif kind == "gb":
                    g_piece(op[1], 1)
                elif kind == "bias":
                    bias_chunk(op[1])

            # preloop: th chunks 0,1 + ph chunk 0.  A1-gated pieces first
            # (th0, ph0), A2-gated th1 last; evictions split scalar/DVE so
            # they pipeline.  Warmup matmuls interleave into the dead PE
            # windows (waiting for x-A2 / evictions) so the HAM activity
            # monitor never sees a ~3.4us idle window and re-throttles.
            def pre_warm(n):
                for _ in range(n):
                    pwt = pw.tile([128, 512], F32, tag="pw", name="warm_mm")
                    nc.tensor.matmul(pwt[:], warm[:, 0:128], warm[:],
                                     start=True, stop=True)

            proj_piece("th", 0, ev="s")
            proj_piece("ph", 0, ev="v")
            pre_warm(7)
            # th1 is the last gate before f(0): evict its two halves on
            # Scalar and DVE in parallel instead of one 0.7us op, and no
            # trailing warmups (they'd sit in the Tensor FIFO ahead of f(0);
            # the th1-eviction idle window is only ~0.9us, below the HAM
            # throttle threshold).
            lo1 = PW
            pp1 = pw.tile([128, 512], F32, tag="pw", name="thp_1")
            for k in range(2):
                nc.tensor.matmul(
                    pp1[:], thw[:, k * CI:(k + 1) * CI],
                    xs[k][:, lo1:lo1 + 512],
                    start=(k == 0), stop=(k == 1),
                )
            nc.scalar.activation(th_sb[:, lo1:lo1 + 256], pp1[:, 0:256],
                                 AF.Identity, bias=thb)
            nc.vector.tensor_scalar_add(th_sb[:, lo1 + 256:lo1 + 512],
                                        pp1[:, 256:512], thb)

            # deferred x DMA chunks
            nc.sync.dma_start(x0[:, 1024:2048], x_d[0:128, 1024:2048])
            nc.scalar.dma_start(x1[:, 1024:2048], x_d[128:256, 1024:2048])
            nc.sync.dma_start(x0[:, 2048:4096], x_d[0:128, 2048:4096])
            nc.scalar.dma_start(x1[:, 2048:4096], x_d[128:256, 2048:4096])
            nc.sync.dma_start(ww[:], ww_d[:])

            # ---------------- per-quarter deferred ops ----------------
            state = {}   # per-quarter: zq, pzt[2], zi, zb, pyt, ynt
            efs = {}

            def zclose(q, s):
                st = state[q]
                pzt = pw.tile([1, 512], F32, tag="pw", name=f"pz_{q}_{s}")
                st["pzt"][s] = pzt
                nc.tensor.matmul(pzt[:], ones_bf[:],
                                 st["zq"][:, s * 512:(s + 1) * 512],
                                 start=True, stop=True)

            def zinv(q, s):
                # fast variant: ~18 correct bits, one DVE op instead of two;
                # Z is in [e^-15, e^25], far from the undefined edge cases.
                st = state[q]
                if s == 0:
                    st["zi"] = zpool.tile([1, QW], F32, tag="zi",
                                          name=f"zi_{q}")
                nc.vector.reciprocal_approx_fast(
                    st["zi"][:, s * 512:(s + 1) * 512], st["pzt"][s][:])

            def bcast(q, s):
                st = state[q]
                if s == 0:
                    st["zb"] = zpool.tile([128, QW], F32, tag="zb",
                                          name=f"zb_{q}")
                nc.gpsimd.partition_broadcast(
                    st["zb"][:, s * 512:(s + 1) * 512],
                    st["zi"][:, s * 512:(s + 1) * 512])

            def mult(q, s):
                st = state[q]
                if s == 0:
                    st["ynt"] = ypool.tile([128, QW], F32R, tag="ynt",
                                           name=f"ynt_{q}")
                nc.vector.tensor_mul(
                    st["ynt"][:, s * 512:(s + 1) * 512],
                    st["pyt"][:, s * 512:(s + 1) * 512],
                    st["zb"][:, s * 512:(s + 1) * 512])

            def wproj(q, chunk, dma_eng=None):
                ob, s2 = divmod(chunk, 2)
                lo = q * QW + s2 * 512
                pwt = pw.tile([128, 512], F32, tag="pw",
                              name=f"pw_{q}_{chunk}")
                nc.tensor.matmul(
                    pwt[:], ww[:, ob * CI:(ob + 1) * CI],
                    state[q]["ynt"][:, s2 * 512:(s2 + 1) * 512],
                    start=True, stop=True)
                ot = opool.tile([128, 512], F32, tag="o", name=f"o_{q}_{chunk}")
                nc.vector.tensor_add(ot[:], pwt[:],
                                     xs[ob][:, lo:lo + 512])
                (dma_eng or nc.sync).dma_start(
                    out_d[ob * 128:(ob + 1) * 128, lo:lo + 512], ot[:])

            # Z accumulation on DVE (bf16 2x mode): binary-counter pairwise
            # tree for j < ZRUN_J, then in-place running adds, so only ONE
            # DVE op remains on the critical path after the last exp of the
            # quarter (a deep tree cascade there costs ~3.5us of tail).
            def tree_push(q, lvl, t_node):
                st = state[q]
                pend = st["pend"]
                if pend.get(lvl) is None:
                    pend[lvl] = t_node
                    return
                a, b = pend.pop(lvl), t_node
                out = ztpool.tile([128, QW], BF16, tag=f"l{lvl}",
                                  name=f"l{lvl}_{q}")
                nc.vector.tensor_add(out[:], a[:], b[:])
                tree_push(q, lvl + 1, out)

            def z_accum(q, j, ef):
                st = state[q]
                if j < ZRUN_J:
                    if j % 2 == 1:
                        pair = ztpool.tile([128, QW], BF16, tag="l0",
                                           name=f"l0_{q}_{j}")
                        nc.vector.tensor_add(pair[:], efs[q * JB + j - 1][:],
                                             ef[:])
                        tree_push(q, 1, pair)
                elif j == ZRUN_J:
                    # merge pending counter partials (sum of 0..23), then run
                    zq = st["zq"] = ztpool.tile([128, QW], BF16, tag="zq",
                                                name=f"zq_{q}")
                    p4, p3 = st["pend"].pop(4), st["pend"].pop(3)
                    nc.vector.tensor_add(zq[:], p4[:], p3[:])
                    nc.vector.tensor_add(zq[:], zq[:], ef[:])
                elif q < NQ - 1 or j < JB - 4:
                    nc.vector.tensor_add(st["zq"][:], st["zq"][:], ef[:])
                else:
                    # last 4 adds of the final quarter in halves: the s0 half
                    # of zq completes right after the last exp, so the tail's
                    # Z-close for s=0 (subtile dep) starts ~0.5us earlier
                    for s in range(2):
                        sl = slice(s * 512, (s + 1) * 512)
                        nc.vector.tensor_add(st["zq"][:, sl], st["zq"][:, sl],
                                             ef[:, sl])

            # ---------------- main flat pipeline ----------------
            for t in range(T + YLAG + 1):
                q, j = divmod(t, JB)
                if t < T:
                    if j == 0:
                        state[q] = {"pzt": [None, None], "pend": {}}
                    st = state[q]
                    i0 = q * QW
                    pft = pf.tile([128, QW], F32, tag="pf", name=f"pf_{t}")
                    for s in range(2):
                        nc.tensor.matmul(
                            pft[:, s * 512:(s + 1) * 512],
                            ph_sb[:, j * 128:(j + 1) * 128],
                            th_sb[:, i0 + s * 512:i0 + (s + 1) * 512],
                            start=True, stop=True)
                    ef = efpool.tile([128, QW], BF16, tag="ef", name=f"ef_{t}")
                    efs[t] = ef
                    nc.scalar.activation(ef[:], pft[:], AF.Exp, bias=aux[:, 5:6])
                    z_accum(q, j, ef)
                    # previous quarter's deferred work
                    if q > 0:
                        if j == ZCLOSE_J[0]:
                            zclose(q - 1, 0)
                        elif j == ZCLOSE_J[1]:
                            zclose(q - 1, 1)
                        elif j == ZINV_J:
                            zinv(q - 1, 0)
                            zinv(q - 1, 1)
                        elif j == BCAST_J[0]:
                            bcast(q - 1, 0)
                        elif j == BCAST_J[1]:
                            bcast(q - 1, 1)
                        elif j == MULT_J:
                            mult(q - 1, 0)
                            mult(q - 1, 1)
                        elif WPROJ_J0 <= j < WPROJ_J0 + 4:
                            wproj(q - 1, j - WPROJ_J0)
                    for op in sched.get(t, []):
                        emit_sched_op(op)
                # trailing y accumulation
                ty = t - YLAG
                if 0 <= ty < T:
                    qy, jy = divmod(ty, JB)
                    if jy == 0:
                        state[qy]["pyt"] = py.tile([128, QW], F32, tag="py",
                                                   name=f"py_{qy}")
                    efy = efs.pop(ty)
                    for s in range(2):
                        nc.tensor.matmul(
                            state[qy]["pyt"][:, s * 512:(s + 1) * 512],
                            gT_sb[:, jy * 128:(jy + 1) * 128],
                            efy[:, s * 512:(s + 1) * 512],
                            start=(jy == 0), stop=(jy == JB - 1))

            # ---------------- last quarter's tail (pipelined by half) ------
            # Dummy matmuls bridge the PE-idle window while DVE/GpSimd run
            # the Z-finalize chain, so the HAM clock gate stays at 2.4 GHz
            # for the W-projection matmuls.
            def pe_dummy(n, i0):
                for i in range(n):
                    pd = pf.tile([128, 512], F32, tag="pf",
                                 name=f"dummy_{i0 + i}")
                    nc.tensor.matmul(pd[:], warm[:, 0:128], warm[:],
                                     start=True, stop=True)

            q = NQ - 1
            pe_dummy(2, 0)
            zclose(q, 0)
            zinv(q, 0)
            bcast(q, 0)
            zclose(q, 1)
            zinv(q, 1)
            bcast(q, 1)
            pe_dummy(12, 2)
            mult(q, 0)
            wproj(q, 0, nc.scalar)   # both s0 chunks before mult(s1);
            wproj(q, 2)              # out-DMAs alternate the two queues
            mult(q, 1)
            wproj(q, 1, nc.scalar)
            wproj(q, 3)

    nc.compile()
    return nc


_CACHE = {}


def _get_nc():
    if "nc" not in _CACHE:
        _CACHE["nc"] = build()
    return _CACHE["nc"]


def _in_maps(x, g_w, g_b, theta_w, theta_b, phi_w, phi_b, W_w, W_b):
    x = np.ascontiguousarray(np.asarray(x, dtype=np.float32))
    wbe = (np.asarray(W_w, np.float32) @ np.asarray(g_b, np.float32)
           + np.asarray(W_b, np.float32))
    common = {
        "thw_t": np.ascontiguousarray(np.asarray(theta_w, np.float32).T),
        "phw_t": np.ascontiguousarray(np.asarray(phi_w, np.float32).T),
        "gw_t": np.ascontiguousarray(np.asarray(g_w, np.float32).T),
        "ww_t": np.ascontiguousarray(np.asarray(W_w, np.float32).T),
        "aux": np.stack(
            [
                np.asarray(theta_b, np.float32),
                np.asarray(phi_b, np.float32),
                wbe[:128],
                wbe[128:],
                np.ones(128, np.float32),
                np.full(128, -40.0, np.float32),
            ],
            axis=1,
        ),
    }
    return [
        {"x": np.ascontiguousarray(x[b].reshape(C, N)), **common}
        for b in range(B)
    ]


def run(in_maps, **kw):
    nc = _get_nc()
    return run_bass_kernel_spmd(nc, in_maps, list(range(B)), **kw)


def kernel(**inputs):
    res = run(_in_maps(**inputs))
    out = np.stack([res.results[b]["out"] for b in range(B)])
    return out.reshape(B, C, H, Wd)
